# revision 56
# baseline (speedup 1.0000x reference)
"""Multi-head attention (b=2, sq=skv=2048, dim=1024, 16 heads x 64) on 8 TRN2
NeuronCores.

Sharding: 2 heads per core (head-parallel), with the matching tensor-parallel
column slice of W_qkv and row slice of W_out.  Each core computes a partial
output projection over its 128 head-dims; the all-reduce of the 8 partials
(+ bias) happens on the host during unshard.

Per-core kernel (bf16 compute, fp32 PSUM accumulation), restructured around
two facts: (1) PE matmul time = streamed free-size only, so the PV matmul is
transposed (stationary = exp-probs tile [k,128q], streaming v [k,65]) to use
all 128 output partitions — half the PE time of the [65,q] layout; (2) ACT
exp time (~141us/core) is the hard floor, so emission is a flat per-step
agenda that keeps exactly one exp per step on ACT while PE fills each step
with scores (critical chain, emitted last in the step), PV, and ~0.4us
projection/out-projection quanta placed by deadline.

The PV accumulator [128q, 4qs, 65] packs 4 accumulation regions in one PSUM
bank: only the first matmul of the bank uses start=True (pending-zero covers
the whole 2KB zero region, so later regions' first write still overwrites).
Normalization: denominator is PV column 64 -> DVE reciprocal + per-partition
tensor_scalar_mul, then a PE transpose back to [d, q] for the out-projection.
"""

import os
import sys

for _p in ("/opt/trn_rl_repo", "/root/.axon_site/_ro/trn_rl_repo"):
    if os.path.isdir(_p) and _p not in sys.path:
        sys.path.append(_p)

from collections import defaultdict

import ml_dtypes
import numpy as np

import concourse.bass as bass  # noqa: F401
import concourse.tile as tile
from concourse import bacc, mybir
from concourse.bass_utils import run_bass_kernel_spmd
from concourse.masks import make_identity

B, SQ, SKV, DIM = 2, 2048, 2048, 1024
HEADS, DH = 16, 64
N_CORES = 8
HPC = HEADS // N_CORES  # heads per core = 2
HD = HPC * DH  # 128 head-dim rows per core
TOK = B * SQ  # 4096
KO = DIM // 128  # 8 contraction chunks of 128
SCALE = DH**-0.5

BF16 = mybir.dt.bfloat16
F32 = mybir.dt.float32

PCHUNK = 512  # token chunk in dram layout
QTILE = 512  # q tile in attention
KTILE = 128  # k tile (scores psum partition dim) == proj jlet width
NKT = SKV // KTILE  # 16
NQT = SQ // QTILE  # 4
NQS = QTILE // 128  # 4 q sub-tiles per q tile
NT = B * NQT * NKT  # 128 attention steps
NCPB = SQ // PCHUNK  # 4 chunks per (batch, tensor)

LA = 4  # exp emitted LA steps behind scores
PVD = 4  # pv emitted PVD steps behind exp

BF = ml_dtypes.bfloat16
Exp = mybir.ActivationFunctionType.Exp


def build():
    nc = bacc.Bacc(
        "TRN2", target_bir_lowering=False, debug=False, num_devices=N_CORES
    )

    NCH = TOK // PCHUNK
    NP = PCHUNK // KTILE  # 4 x 128-token pieces per chunk (piece-contiguous)
    xqt_d = nc.dram_tensor("xqt", [NCH, 128, NP, KO, KTILE], BF16, kind="ExternalInput")
    xkvt_d = nc.dram_tensor("xkvt", [NCH, 128, NP, KO, KTILE], BF16, kind="ExternalInput")
    wq_d = nc.dram_tensor("wq", [DIM, HD], BF16, kind="ExternalInput")
    wk_d = nc.dram_tensor("wk", [DIM, HD], BF16, kind="ExternalInput")
    wv_d = nc.dram_tensor("wv", [DIM, HD], BF16, kind="ExternalInput")
    wout_d = nc.dram_tensor("wout", [HD, DIM], BF16, kind="ExternalInput")
    # [b*NQT + qt, partition p, q-sub tt, dim]: token = (b, qt*512 + tt*128 + p)
    out_d = nc.dram_tensor("out", [B * NQT, 128, NQS, DIM], BF16, kind="ExternalOutput")

    xqt = xqt_d.ap()
    xkvt = xkvt_d.ap()

    with tile.TileContext(nc) as tc:
        with (
            tc.tile_pool(name="persist", bufs=1) as persist,
            tc.tile_pool(name="xin", bufs=6) as xin,
            tc.tile_pool(name="exps", bufs=12) as exps,
            tc.tile_pool(name="obp", bufs=4) as obp,
            tc.tile_pool(name="ost", bufs=2) as ost,
            tc.tile_pool(name="spsum", bufs=2, space="PSUM") as spsum,
            tc.tile_pool(name="accp", bufs=2, space="PSUM") as accp,
            tc.tile_pool(name="miscp", bufs=2, space="PSUM") as miscp,
        ):
            # --- weights / constants ---
            wq_sb = persist.tile([128, KO, HD], BF16, tag="wq")
            nc.gpsimd.dma_start(wq_sb[:], wq_d.ap().rearrange("(ko p) m -> p ko m", p=128))
            wk_sb = persist.tile([128, KO, HD], BF16, tag="wk")
            nc.gpsimd.dma_start(wk_sb[:], wk_d.ap().rearrange("(ko p) m -> p ko m", p=128))
            wv_sb = persist.tile([128, KO, HD], BF16, tag="wv")
            nc.gpsimd.dma_start(wv_sb[:], wv_d.ap().rearrange("(ko p) m -> p ko m", p=128))
            wout_sb = persist.tile([HD, DIM], BF16, tag="wout")
            # dma'd from the agenda (step 5) so the first x chunks go ahead
            # of it in the DGE ring

            ident = persist.tile([128, DH], BF16, tag="ident")
            make_identity(nc, ident[0:DH, :])
            make_identity(nc, ident[DH : 2 * DH, :])
            ident128 = persist.tile([128, 128], BF16, tag="id128")
            make_identity(nc, ident128[:])
            # prefetch the exp table set during the head DMAs
            dummy = persist.tile([1, 8], F32, tag="dummy")
            nc.vector.memset(dummy[:], 0.0)
            nc.scalar.activation(dummy[:], dummy[:], Exp)

            qt_sb, kt_sb, vt_sb, vnat, outT = {}, {}, {}, {}, {}
            for b in range(B):
                qt_sb[b] = persist.tile([HD, SQ], BF16, tag=f"qt{b}", name=f"qt{b}")
                kt_sb[b] = persist.tile([HD, SKV], BF16, tag=f"kt{b}", name=f"kt{b}")
                vt_sb[b] = persist.tile([HD, SKV], BF16, tag=f"vt{b}", name=f"vt{b}")
                vnat[b] = persist.tile(
                    [128, HPC, NKT, DH + 1], BF16, tag=f"vn{b}", name=f"vn{b}"
                )
                outT[b] = persist.tile([HD, SQ], BF16, tag=f"ot{b}", name=f"ot{b}")
                nc.vector.memset(vnat[b][:, :, :, DH], 1.0)

            # --- emitter state ---
            xts = {}  # (tensor, b, chunk) -> x tile
            sps = {}  # t -> scores psum tile
            exs = {}  # t -> exp sbuf tile
            accs = {}  # (qtg, h) -> pv psum accumulator
            obufs = {}  # (qtg, h) -> normalized [128q, 4, 64] bf16
            obs = {}  # qtg -> out-projection sbuf tile

            def load_x(tensor, b, c, x_ap, eng=None):
                def go():
                    xt = xin.tile([128, NP, KO, KTILE], BF16, tag="x", name="xt")
                    (eng or nc.gpsimd).dma_start(xt[:], x_ap[b * NCPB + c])
                    xts[(tensor, b, c)] = xt

                return go

            def proj_jlet(dst_sb, w_sb, tensor, b, j):
                # project tokens [j*128, (j+1)*128) of chunk c = j//4
                def go():
                    c, sub = divmod(j, 4)
                    xt = xts[(tensor, b, c)]
                    rhs = [xt[:, sub, ko, :] for ko in range(KO)]
                    ps = miscp.tile([128, KTILE], F32, tag="m", name="projp")
                    for ko in range(KO):
                        nc.tensor.matmul(
                            ps[:],
                            w_sb[:, ko, :],
                            rhs[ko],
                            start=(ko == 0),
                            stop=(ko == KO - 1),
                        )
                    nc.vector.tensor_copy(
                        dst_sb[:, j * KTILE : (j + 1) * KTILE], ps[:]
                    )

                return go

            def vnat_group(b, jg):
                def go():
                    for h in range(HPC):
                        tp = miscp.tile([128, 4, DH], BF16, tag="m", name="vtp")
                        for i in range(4):
                            j = jg * 4 + i
                            nc.tensor.transpose(
                                tp[:, i, :],
                                vt_sb[b][
                                    h * DH : (h + 1) * DH,
                                    j * KTILE : (j + 1) * KTILE,
                                ],
                                ident[h * DH : (h + 1) * DH, :],
                            )
                        nc.vector.tensor_copy(
                            vnat[b][:, h, jg * 4 : (jg + 1) * 4, 0:DH], tp[:]
                        )

                return go

            def t_coords(t):
                b, r = divmod(t, NQT * NKT)
                qt, j = divmod(r, NKT)
                return b, qt, j

            def do_scores(t):
                b, qt, j = t_coords(t)
                k_sl = slice(j * KTILE, (j + 1) * KTILE)
                sp = spsum.tile([128, HPC, QTILE], F32, tag="s", name="sp")
                sps[t] = sp
                # 128-free strips: the two heads run as concurrent row-tiles,
                # and the post-wait p-state penalty lands on a short matmul
                if t < 8 or t >= NT - 8:
                    # head/tail run at cold clock: one wide matmul per head
                    # takes the p-state hit once instead of per strip
                    q_sl = slice(qt * QTILE, (qt + 1) * QTILE)
                    for h in range(HPC):
                        h_sl = slice(h * DH, (h + 1) * DH)
                        nc.tensor.matmul(
                            sp[:, h, :],
                            kt_sb[b][h_sl, k_sl],
                            qt_sb[b][h_sl, q_sl],
                            start=True,
                            stop=True,
                        )
                    return
                # one accumulation group per head-bank (start first / stop
                # last strip) so the exp waits on 2 semaphores, not 8
                for qs in range(NQS):
                    q_sl = slice(qt * QTILE + qs * 128, qt * QTILE + (qs + 1) * 128)
                    for h in range(HPC):
                        h_sl = slice(h * DH, (h + 1) * DH)
                        nc.tensor.matmul(
                            sp[:, h, qs * 128 : (qs + 1) * 128],
                            kt_sb[b][h_sl, k_sl],
                            qt_sb[b][h_sl, q_sl],
                            start=(qs == 0),
                            stop=(qs == NQS - 1),
                        )

            def do_exp(t):
                sp = sps.pop(t)
                ex = exps.tile([128, HPC, QTILE], BF16, tag="e", name="ex")
                nc.scalar.activation(ex[:], sp[:], Exp, scale=SCALE)
                exs[t] = ex

            def do_pv(t):
                b, qt, j = t_coords(t)
                qtg = t // NKT
                ex = exs.pop(t)
                if j == 0:
                    for h in range(HPC):
                        accs[(qtg, h)] = accp.tile(
                            [128, NQS, DH + 1], F32, tag="acc", name="acc"
                        )
                for h in range(HPC):
                    acc = accs[(qtg, h)]
                    for qs in range(NQS):
                        # one accumulation group per psum bank: only the very
                        # first write starts it (pending-zero spans the bank)
                        nc.tensor.matmul(
                            acc[:, qs, :],
                            ex[:, h, qs * 128 : (qs + 1) * 128],
                            vnat[b][:, h, j, :],
                            start=(j == 0 and qs == 0),
                            stop=(j == NKT - 1 and qs == NQS - 1),
                        )
                if j == NKT - 1:
                    for h in range(HPC):
                        acc = accs[(qtg, h)]
                        rec = obp.tile([128, NQS], F32, tag="rc", name="rec")
                        nc.vector.reciprocal(rec[:], acc[:, :, DH])
                        ob = obp.tile([128, NQS, DH], BF16, tag="ob", name="obuf")
                        for qs in range(NQS):
                            nc.vector.tensor_scalar_mul(
                                ob[:, qs, :],
                                acc[:, qs, 0:DH],
                                rec[:, qs : qs + 1],
                            )
                        obufs[(qtg, h)] = ob

            def flush2(qtg):
                # transpose normalized o [128q, 64] back to [64, 128q] and park
                # it in outT for the out-projection
                def go():
                    b, qt = divmod(qtg, NQT)
                    for h in range(HPC):
                        ob = obufs.pop((qtg, h))
                        oT = accp.tile([DH, NQS, 128], BF16, tag="acc", name="oT")
                        for qs in range(NQS):
                            nc.tensor.transpose(
                                oT[:, qs, :], ob[:, qs, :], ident128[:]
                            )
                        nc.vector.tensor_copy(
                            outT[b][
                                h * DH : (h + 1) * DH,
                                qt * QTILE : (qt + 1) * QTILE,
                            ],
                            oT[:].rearrange("p a b -> p (a b)"),
                        )

                return go

            def outproj_mm(qtg, tt, nt):
                def go():
                    b, qt = divmod(qtg, NQT)
                    if qtg not in obs:
                        obs[qtg] = ost.tile(
                            [128, NQS, DIM], BF16, tag="o", name="ob"
                        )
                    t_sl = slice((qt * NQS + tt) * 128, (qt * NQS + tt + 1) * 128)
                    ps = miscp.tile([128, 512], F32, tag="m", name="projo")
                    # 4 x 128-free matmuls pipeline tighter than 1 x 512-free
                    for qq in range(4):
                        nc.tensor.matmul(
                            ps[:, qq * 128 : (qq + 1) * 128],
                            outT[b][:, t_sl],
                            wout_sb[:, nt * 512 + qq * 128 : nt * 512 + (qq + 1) * 128],
                            start=(qq == 0),
                            stop=(qq == 3),
                        )
                    dst = obs[qtg][:, tt, nt * 512 : (nt + 1) * 512]
                    if qtg == 6 and tt >= 2:
                        # post-attention steps: ACT is free after the last exp
                        nc.scalar.copy(dst, ps[:])
                    else:
                        nc.vector.tensor_copy(dst, ps[:])

                return go

            def outproj_dma(qtg):
                def go():
                    nc.gpsimd.dma_start(out_d.ap()[qtg], obs.pop(qtg)[:])

                return go

            # ---------------- agenda ----------------
            agenda = defaultdict(list)

            def at(s, prio, fn):
                agenda[s].append((prio, fn))

            # core attention stream: pv (prio 1), exp (2), scores last (8)
            # except the first two steps where nothing gates scores
            for t in range(NT):
                at(t, 0 if t < 2 else 8, lambda t=t: do_scores(t))
                at(t + LA, 2, lambda t=t: do_exp(t))
                j = t % NKT
                if t >= NT - NKT and j >= 7:
                    # drain the tail: shrink the pv lag towards the end so
                    # little PV work remains after the last exp
                    lag = 7 if j <= 8 else 6
                else:
                    lag = LA + PVD + (2 if j == 0 else (1 if j == 1 else 0))
                at(t + lag, 1, lambda t=t: do_pv(t))

            # flush + out-projection per q-tile group (last one handled in
            # the epilogue below).  Quanta go on every other step to avoid
            # overloading any single step's PE budget.
            for qtg in range(B * NQT - 1):
                base = qtg * NKT
                at(base + 25, 3, flush2(qtg))
                q8 = 0
                for tt in range(NQS):
                    for nt in range(2):
                        if qtg < 5:
                            # qtg3/4 on odd steps: b1 q-jlets hold the evens
                            s = base + 26 + (qtg >= 3) + 2 * q8
                        elif qtg == 5:
                            # late half warms the PE through the last scores
                            s = base + 26 + 2 * q8 if q8 < 4 else base + 31 + 2 * q8
                        else:
                            # qtg6's quanta run after the last scores
                            s = base + 32 + q8
                        at(s, 4, outproj_mm(qtg, tt, nt))
                        q8 += 1
                at(base + {5: 47, 6: 41}.get(qtg, 42), 5, outproj_dma(qtg))

            # ---- projection / load filler (placed by deadline) ----
            # b0: scores(qt0, j) at step j needs kt jlet j; vnat g by pv step
            for j in range(1, NKT):
                at(j - 1, 4, proj_jlet(kt_sb[0], wk_sb, "kv", 0, j))
            for j in range(2, NKT):
                at(j - 2, 5, proj_jlet(vt_sb[0], wv_sb, "kv", 0, j))
            for g in range(4):
                at(4 * g + 5, 5, vnat_group(0, g))
            at(0, 0, lambda: nc.gpsimd.dma_start(wout_sb[:], wout_d.ap()))
            at(2, 0, load_x("kv", 0, 2, xkvt))
            at(4, 0, load_x("q", 0, 1, xqt))
            at(6, 0, load_x("kv", 0, 3, xkvt))
            at(18, 0, load_x("q", 0, 2, xqt))
            at(34, 0, load_x("q", 0, 3, xqt))
            for sub in range(4):
                at(11 + sub, 4, proj_jlet(qt_sb[0], wq_sb, "q", 0, 4 + sub))
            for sub in range(4):
                at(19 + 2 * sub, 4, proj_jlet(qt_sb[0], wq_sb, "q", 0, 8 + sub))
                at(35 + 2 * sub, 4, proj_jlet(qt_sb[0], wq_sb, "q", 0, 12 + sub))
            # b1 loads + projections, woven through b0's steady state
            for c in range(NCPB):
                at(24 + 8 * c, 0, load_x("kv", 1, c, xkvt))
            for j in range(NKT):
                at(28 + 2 * j, 4, proj_jlet(kt_sb[1], wk_sb, "kv", 1, j))
            for j in range(NKT):
                at(44 + j, 5, proj_jlet(vt_sb[1], wv_sb, "kv", 1, j))
            for g in range(4):
                at(60 + g, 4, vnat_group(1, g))
            for c in range(NCPB):
                at(48 + 14 * c, 0, load_x("q", 1, c, xqt))
                for sub in range(4):
                    s = 56 + sub if c == 0 else 54 + 16 * c + 2 * sub
                    at(s, 4, proj_jlet(qt_sb[1], wq_sb, "q", 1, 4 * c + sub))

            # ---------------- prologue: one DGE ring, strict priority order
            # (splitting rings splits the fixed aggregate bandwidth away
            # from the critical kv00+q00 prefix)
            load_x("q", 0, 0, xqt)()
            load_x("kv", 0, 0, xkvt)()
            load_x("kv", 0, 1, xkvt)()
            # spin the PE on junk matmuls while the first chunks are in
            # flight: keeps the clock at full p-state so the prologue
            # projections don't run cold.  The source tile is DVE-memset so
            # the spin starts right after the startup barrier (~7us), not
            # after the gpsimd identity setup (~12us).
            wsrc = persist.tile([128, 32], BF16, tag="wsrc")
            nc.vector.memset(wsrc[:], 0.25)
            warm_ps = miscp.tile([128, 32], F32, tag="m", name="warm")
            for _ in range(220):
                nc.tensor.matmul(
                    warm_ps[0:32, :], wsrc[:], wsrc[:],
                    start=True, stop=True,
                )
            for sub in range(4):
                proj_jlet(qt_sb[0], wq_sb, "q", 0, sub)()
            proj_jlet(kt_sb[0], wk_sb, "kv", 0, 0)()
            at(0, 3, proj_jlet(vt_sb[0], wv_sb, "kv", 0, 0))
            at(0, 3, proj_jlet(vt_sb[0], wv_sb, "kv", 0, 1))

            # ---------------- run the agenda ----------------
            for s in sorted(agenda):
                for _, fn in sorted(agenda[s], key=lambda pf: pf[0]):
                    fn()

            # ---------------- epilogue: last q-tile group ----------------
            qtg = B * NQT - 1
            b, qt = divmod(qtg, NQT)
            obs[qtg] = ost.tile([128, NQS, DIM], BF16, tag="o", name="ob")
            oTs = {}
            for h in range(HPC):
                oTs[h] = accp.tile([DH, NQS, 128], BF16, tag="acc", name="oT")
            for qs in range(NQS):
                for h in range(HPC):
                    nc.tensor.transpose(
                        oTs[h][:, qs, :], obufs[(qtg, h)][:, qs, :], ident128[:]
                    )
                    nc.vector.tensor_copy(
                        outT[b][
                            h * DH : (h + 1) * DH,
                            qt * QTILE + qs * 128 : qt * QTILE + (qs + 1) * 128,
                        ],
                        oTs[h][:, qs, :],
                    )
                t_sl = slice((qt * NQS + qs) * 128, (qt * NQS + qs + 1) * 128)
                for nt in range(2):
                    ps = miscp.tile([128, 512], F32, tag="m", name="projo")
                    for qq in range(4):
                        nc.tensor.matmul(
                            ps[:, qq * 128 : (qq + 1) * 128],
                            outT[b][:, t_sl],
                            wout_sb[:, nt * 512 + qq * 128 : nt * 512 + (qq + 1) * 128],
                            start=(qq == 0),
                            stop=(qq == 3),
                        )
                    dst = obs[qtg][:, qs, nt * 512 : (nt + 1) * 512]
                    if nt == 0:
                        nc.scalar.copy(dst, ps[:])
                    else:
                        nc.vector.tensor_copy(dst, ps[:])
                nc.gpsimd.dma_start(
                    out_d.ap()[qtg].rearrange("p t d -> t p d")[qs],
                    obs[qtg][:, qs, :],
                )
            obs.pop(qtg)

    nc.compile()
    return nc


def make_in_maps(x_q, x_kv, W_qkv, W_out):
    x_q = np.asarray(x_q, dtype=np.float32)
    x_kv = np.asarray(x_kv, dtype=np.float32)
    W_qkv = np.asarray(W_qkv, dtype=np.float32)
    W_out = np.asarray(W_out, dtype=np.float32)

    def chunk_tile(x):
        # [TOK, DIM] -> [n_chunks, 128, 4, KO, 128]: piece-contiguous layout,
        # token = c*512 + piece*128 + t, D = ko*128 + partition
        xt = x.reshape(TOK, DIM).T.reshape(KO, 128, TOK // PCHUNK, PCHUNK // KTILE, KTILE)
        return np.ascontiguousarray(xt.transpose(2, 1, 3, 0, 4)).astype(BF)

    xqt = chunk_tile(x_q)
    xkvt = chunk_tile(x_kv)

    in_maps = []
    for c in range(N_CORES):
        cs = slice(c * HD, (c + 1) * HD)
        in_maps.append(
            {
                "xqt": xqt,
                "xkvt": xkvt,
                "wq": np.ascontiguousarray(W_qkv[:, cs]).astype(BF),
                "wk": np.ascontiguousarray(W_qkv[:, 1024:][:, cs]).astype(BF),
                "wv": np.ascontiguousarray(W_qkv[:, 2048:][:, cs]).astype(BF),
                "wout": np.ascontiguousarray(W_out[cs, :]).astype(BF),
            }
        )
    return in_maps


def combine(partials, b_out):
    """Sum the 8 per-core partial projections and add the bias."""
    acc = np.zeros((B * NQT, 128, NQS, DIM), dtype=np.float32)
    for p in partials:
        acc += np.asarray(p, dtype=np.float32)
    # [bq, p, tt, d] -> token (bq, tt, p)
    acc = acc.transpose(0, 2, 1, 3).reshape(B, SQ, DIM)
    acc += np.asarray(b_out, dtype=np.float32)
    return acc


_STATE = {}


def _get_nc():
    if "nc" not in _STATE:
        _STATE["nc"] = build()
    return _STATE["nc"]


def run(x_q, x_kv, W_qkv, W_out, b_out, trace=False):
    nc = _get_nc()
    in_maps = make_in_maps(x_q, x_kv, W_qkv, W_out)
    res = run_bass_kernel_spmd(nc, in_maps, list(range(N_CORES)), trace=trace)
    out = combine([r["out"] for r in res.results], b_out)
    return out, res


def kernel(x_q, x_kv, W_qkv, W_out, b_out):
    out, _ = run(x_q, x_kv, W_qkv, W_out, b_out, trace=False)
    return out


# revision 59
# speedup vs baseline: 1.1865x; 1.1865x over previous
"""Multi-head attention (b=2, sq=skv=2048, dim=1024, 16 heads x 64) on 8 TRN2
NeuronCores.

Sharding: 2 heads per core (head-parallel), with the matching tensor-parallel
column slice of W_qkv and row slice of W_out.  Each core computes a partial
output projection over its 128 head-dims; the all-reduce of the 8 partials
(+ bias) happens on the host during unshard.

Per-core kernel (bf16 compute, fp32 PSUM accumulation), restructured around
two facts: (1) PE matmul time = streamed free-size only, so the PV matmul is
transposed (stationary = exp-probs tile [k,128q], streaming v [k,65]) to use
all 128 output partitions — half the PE time of the [65,q] layout; (2) ACT
exp time (~141us/core) is the hard floor, so emission is a flat per-step
agenda that keeps exactly one exp per step on ACT while PE fills each step
with scores (critical chain, emitted last in the step), PV, and ~0.4us
projection/out-projection quanta placed by deadline.

The PV accumulator [128q, 4qs, 65] packs 4 accumulation regions in one PSUM
bank: only the first matmul of the bank uses start=True (pending-zero covers
the whole 2KB zero region, so later regions' first write still overwrites).
Normalization: denominator is PV column 64 -> DVE reciprocal + per-partition
tensor_scalar_mul, then a PE transpose back to [d, q] for the out-projection.
"""

import os
import sys

for _p in ("/opt/trn_rl_repo", "/root/.axon_site/_ro/trn_rl_repo"):
    if os.path.isdir(_p) and _p not in sys.path:
        sys.path.append(_p)

from collections import defaultdict

import ml_dtypes
import numpy as np

import concourse.bass as bass  # noqa: F401
import concourse.tile as tile
from concourse import bacc, mybir
from concourse.bass_utils import run_bass_kernel_spmd
from concourse.masks import make_identity

B, SQ, SKV, DIM = 2, 2048, 2048, 1024
HEADS, DH = 16, 64
N_CORES = 8
HPC = HEADS // N_CORES  # heads per core = 2
HD = HPC * DH  # 128 head-dim rows per core
TOK = B * SQ  # 4096
KO = DIM // 128  # 8 contraction chunks of 128
SCALE = DH**-0.5

BF16 = mybir.dt.bfloat16
F32 = mybir.dt.float32

PCHUNK = 512  # token chunk in dram layout
QTILE = 512  # q tile in attention
KTILE = 128  # k tile (scores psum partition dim) == proj jlet width
NKT = SKV // KTILE  # 16
NQT = SQ // QTILE  # 4
NQS = QTILE // 128  # 4 q sub-tiles per q tile
NT = B * NQT * NKT  # 128 attention steps
NCPB = SQ // PCHUNK  # 4 chunks per (batch, tensor)

LA = 4  # exp emitted LA steps behind scores
PVD = 4  # pv emitted PVD steps behind exp

BF = ml_dtypes.bfloat16
Exp = mybir.ActivationFunctionType.Exp


def build():
    nc = bacc.Bacc(
        "TRN2", target_bir_lowering=False, debug=False, num_devices=N_CORES
    )

    NCH = TOK // PCHUNK
    NP = PCHUNK // KTILE  # 4 x 128-token pieces per chunk (piece-contiguous)
    xqt_d = nc.dram_tensor("xqt", [NCH, 128, NP, KO, KTILE], BF16, kind="ExternalInput")
    xkvt_d = nc.dram_tensor("xkvt", [NCH, 128, NP, KO, KTILE], BF16, kind="ExternalInput")
    wq_d = nc.dram_tensor("wq", [DIM, HD], BF16, kind="ExternalInput")
    wk_d = nc.dram_tensor("wk", [DIM, HD], BF16, kind="ExternalInput")
    wv_d = nc.dram_tensor("wv", [DIM, HD], BF16, kind="ExternalInput")
    wout_d = nc.dram_tensor("wout", [HD, DIM], BF16, kind="ExternalInput")
    # [b*NQT + qt, partition p, q-sub tt, dim]: token = (b, qt*512 + tt*128 + p)
    out_d = nc.dram_tensor("out", [B * NQT, 128, NQS, DIM], BF16, kind="ExternalOutput")

    xqt = xqt_d.ap()
    xkvt = xkvt_d.ap()

    with tile.TileContext(nc) as tc:
        with (
            tc.tile_pool(name="persist", bufs=1) as persist,
            tc.tile_pool(name="xin", bufs=6) as xin,
            tc.tile_pool(name="exps", bufs=12) as exps,
            tc.tile_pool(name="obp", bufs=4) as obp,
            tc.tile_pool(name="ost", bufs=2) as ost,
            tc.tile_pool(name="spsum", bufs=2, space="PSUM") as spsum,
            tc.tile_pool(name="accp", bufs=2, space="PSUM") as accp,
            tc.tile_pool(name="miscp", bufs=2, space="PSUM") as miscp,
        ):
            # --- weights / constants ---
            wq_sb = persist.tile([128, KO, HD], BF16, tag="wq")
            nc.gpsimd.dma_start(wq_sb[:], wq_d.ap().rearrange("(ko p) m -> p ko m", p=128))
            wk_sb = persist.tile([128, KO, HD], BF16, tag="wk")
            nc.gpsimd.dma_start(wk_sb[:], wk_d.ap().rearrange("(ko p) m -> p ko m", p=128))
            wv_sb = persist.tile([128, KO, HD], BF16, tag="wv")
            nc.gpsimd.dma_start(wv_sb[:], wv_d.ap().rearrange("(ko p) m -> p ko m", p=128))
            wout_sb = persist.tile([HD, DIM], BF16, tag="wout")
            # dma'd from the agenda (step 5) so the first x chunks go ahead
            # of it in the DGE ring

            ident = persist.tile([128, DH], BF16, tag="ident")
            make_identity(nc, ident[0:DH, :])
            make_identity(nc, ident[DH : 2 * DH, :])
            ident128 = persist.tile([128, 128], BF16, tag="id128")
            make_identity(nc, ident128[:])
            # prefetch the exp table set during the head DMAs
            dummy = persist.tile([1, 8], F32, tag="dummy")
            nc.vector.memset(dummy[:], 0.0)
            nc.scalar.activation(dummy[:], dummy[:], Exp)

            qt_sb, kt_sb, vt_sb, vnat, outT = {}, {}, {}, {}, {}
            for b in range(B):
                qt_sb[b] = persist.tile([HD, SQ], BF16, tag=f"qt{b}", name=f"qt{b}")
                kt_sb[b] = persist.tile([HD, SKV], BF16, tag=f"kt{b}", name=f"kt{b}")
                vt_sb[b] = persist.tile([HD, SKV], BF16, tag=f"vt{b}", name=f"vt{b}")
                vnat[b] = persist.tile(
                    [128, HPC, NKT, DH + 1], BF16, tag=f"vn{b}", name=f"vn{b}"
                )
                outT[b] = persist.tile([HD, SQ], BF16, tag=f"ot{b}", name=f"ot{b}")
                nc.vector.memset(vnat[b][:, :, :, DH], 1.0)

            # --- emitter state ---
            xts = {}  # (tensor, b, chunk) -> x tile
            sps = {}  # t -> scores psum tile
            exs = {}  # t -> exp sbuf tile
            accs = {}  # (qtg, h) -> pv psum accumulator
            obufs = {}  # (qtg, h) -> normalized [128q, 4, 64] bf16
            obs = {}  # qtg -> out-projection sbuf tile

            def load_x(tensor, b, c, x_ap, eng=None):
                def go():
                    xt = xin.tile([128, NP, KO, KTILE], BF16, tag="x", name="xt")
                    (eng or nc.gpsimd).dma_start(xt[:], x_ap[b * NCPB + c])
                    xts[(tensor, b, c)] = xt

                return go

            def proj_jlet(dst_sb, w_sb, tensor, b, j):
                # project tokens [j*128, (j+1)*128) of chunk c = j//4
                def go():
                    c, sub = divmod(j, 4)
                    xt = xts[(tensor, b, c)]
                    rhs = [xt[:, sub, ko, :] for ko in range(KO)]
                    ps = miscp.tile([128, KTILE], F32, tag="m", name="projp")
                    for ko in range(KO):
                        nc.tensor.matmul(
                            ps[:],
                            w_sb[:, ko, :],
                            rhs[ko],
                            start=(ko == 0),
                            stop=(ko == KO - 1),
                        )
                    nc.vector.tensor_copy(
                        dst_sb[:, j * KTILE : (j + 1) * KTILE], ps[:]
                    )

                return go

            def vnat_group(b, jg):
                def go():
                    for h in range(HPC):
                        tp = miscp.tile([128, 4, DH], BF16, tag="m", name="vtp")
                        for i in range(4):
                            j = jg * 4 + i
                            nc.tensor.transpose(
                                tp[:, i, :],
                                vt_sb[b][
                                    h * DH : (h + 1) * DH,
                                    j * KTILE : (j + 1) * KTILE,
                                ],
                                ident[h * DH : (h + 1) * DH, :],
                            )
                        nc.vector.tensor_copy(
                            vnat[b][:, h, jg * 4 : (jg + 1) * 4, 0:DH], tp[:]
                        )

                return go

            def t_coords(t):
                b, r = divmod(t, NQT * NKT)
                qt, j = divmod(r, NKT)
                return b, qt, j

            def do_scores(t):
                b, qt, j = t_coords(t)
                k_sl = slice(j * KTILE, (j + 1) * KTILE)
                sp = spsum.tile([128, HPC, QTILE], F32, tag="s", name="sp")
                sps[t] = sp
                # 128-free strips: the two heads run as concurrent row-tiles,
                # and the post-wait p-state penalty lands on a short matmul
                if t < 8 or t >= NT - 8:
                    # head/tail run at cold clock: one wide matmul per head
                    # takes the p-state hit once instead of per strip
                    q_sl = slice(qt * QTILE, (qt + 1) * QTILE)
                    for h in range(HPC):
                        h_sl = slice(h * DH, (h + 1) * DH)
                        nc.tensor.matmul(
                            sp[:, h, :],
                            kt_sb[b][h_sl, k_sl],
                            qt_sb[b][h_sl, q_sl],
                            start=True,
                            stop=True,
                        )
                    return
                # one accumulation group per head-bank (start first / stop
                # last strip) so the exp waits on 2 semaphores, not 8
                for qs in range(NQS):
                    q_sl = slice(qt * QTILE + qs * 128, qt * QTILE + (qs + 1) * 128)
                    for h in range(HPC):
                        h_sl = slice(h * DH, (h + 1) * DH)
                        nc.tensor.matmul(
                            sp[:, h, qs * 128 : (qs + 1) * 128],
                            kt_sb[b][h_sl, k_sl],
                            qt_sb[b][h_sl, q_sl],
                            start=(qs == 0),
                            stop=(qs == NQS - 1),
                        )

            def do_exp(t):
                sp = sps.pop(t)
                ex = exps.tile([128, HPC, QTILE], BF16, tag="e", name="ex")
                nc.scalar.activation(ex[:], sp[:], Exp, scale=SCALE)
                exs[t] = ex

            def do_pv(t):
                b, qt, j = t_coords(t)
                qtg = t // NKT
                ex = exs.pop(t)
                if j == 0:
                    for h in range(HPC):
                        accs[(qtg, h)] = accp.tile(
                            [128, NQS, DH + 1], F32, tag="acc", name="acc"
                        )
                for h in range(HPC):
                    acc = accs[(qtg, h)]
                    for qs in range(NQS):
                        # one accumulation group per psum bank: only the very
                        # first write starts it (pending-zero spans the bank)
                        nc.tensor.matmul(
                            acc[:, qs, :],
                            ex[:, h, qs * 128 : (qs + 1) * 128],
                            vnat[b][:, h, j, :],
                            start=(j == 0 and qs == 0),
                            stop=(j == NKT - 1 and qs == NQS - 1),
                        )
                if j == NKT - 1:
                    for h in range(HPC):
                        acc = accs[(qtg, h)]
                        rec = obp.tile([128, NQS], F32, tag="rc", name="rec")
                        nc.vector.reciprocal(rec[:], acc[:, :, DH])
                        ob = obp.tile([128, NQS, DH], BF16, tag="ob", name="obuf")
                        for qs in range(NQS):
                            if qtg == B * NQT - 1:
                                # ACT is idle after the last exp: normalize
                                # there to shorten the tail's DVE chain
                                nc.scalar.activation(
                                    ob[:, qs, :],
                                    acc[:, qs, 0:DH],
                                    mybir.ActivationFunctionType.Copy,
                                    scale=rec[:, qs : qs + 1],
                                )
                            else:
                                nc.vector.tensor_scalar_mul(
                                    ob[:, qs, :],
                                    acc[:, qs, 0:DH],
                                    rec[:, qs : qs + 1],
                                )
                        obufs[(qtg, h)] = ob

            def flush2(qtg):
                # transpose normalized o [128q, 64] back to [64, 128q] and park
                # it in outT for the out-projection
                def go():
                    b, qt = divmod(qtg, NQT)
                    for h in range(HPC):
                        ob = obufs.pop((qtg, h))
                        oT = accp.tile([DH, NQS, 128], BF16, tag="acc", name="oT")
                        for qs in range(NQS):
                            nc.tensor.transpose(
                                oT[:, qs, :], ob[:, qs, :], ident128[:]
                            )
                        nc.vector.tensor_copy(
                            outT[b][
                                h * DH : (h + 1) * DH,
                                qt * QTILE : (qt + 1) * QTILE,
                            ],
                            oT[:].rearrange("p a b -> p (a b)"),
                        )

                return go

            def outproj_mm(qtg, tt, nt):
                def go():
                    b, qt = divmod(qtg, NQT)
                    if qtg not in obs:
                        obs[qtg] = ost.tile(
                            [128, NQS, DIM], BF16, tag="o", name="ob"
                        )
                    t_sl = slice((qt * NQS + tt) * 128, (qt * NQS + tt + 1) * 128)
                    ps = miscp.tile([128, 512], F32, tag="m", name="projo")
                    # 4 x 128-free matmuls pipeline tighter than 1 x 512-free
                    for qq in range(4):
                        nc.tensor.matmul(
                            ps[:, qq * 128 : (qq + 1) * 128],
                            outT[b][:, t_sl],
                            wout_sb[:, nt * 512 + qq * 128 : nt * 512 + (qq + 1) * 128],
                            start=(qq == 0),
                            stop=(qq == 3),
                        )
                    dst = obs[qtg][:, tt, nt * 512 : (nt + 1) * 512]
                    if qtg == 6 and tt >= 2:
                        # post-attention steps: ACT is free after the last exp
                        nc.scalar.copy(dst, ps[:])
                    else:
                        nc.vector.tensor_copy(dst, ps[:])

                return go

            def outproj_dma(qtg):
                def go():
                    nc.gpsimd.dma_start(out_d.ap()[qtg], obs.pop(qtg)[:])

                return go

            # ---------------- agenda ----------------
            agenda = defaultdict(list)

            def at(s, prio, fn):
                agenda[s].append((prio, fn))

            # core attention stream: pv (prio 1), exp (2), scores last (8)
            # except the first two steps where nothing gates scores
            for t in range(NT):
                at(t, 0 if t < 2 else 8, lambda t=t: do_scores(t))
                at(t + LA, 2, lambda t=t: do_exp(t))
                j = t % NKT
                if t >= NT - NKT and j >= 7:
                    # drain the tail: shrink the pv lag towards the end so
                    # little PV work remains after the last exp
                    lag = 7 if j <= 8 else (6 if j <= 9 else 5)
                else:
                    lag = LA + PVD + (2 if j == 0 else (1 if j == 1 else 0))
                at(t + lag, 1, lambda t=t: do_pv(t))

            # flush + out-projection per q-tile group (last one handled in
            # the epilogue below).  Quanta go on every other step to avoid
            # overloading any single step's PE budget.
            for qtg in range(B * NQT - 1):
                base = qtg * NKT
                at(base + 25, 3, flush2(qtg))
                q8 = 0
                for tt in range(NQS):
                    for nt in range(2):
                        if qtg < 5:
                            # qtg3/4 on odd steps: b1 q-jlets hold the evens
                            s = base + 26 + (qtg >= 3) + 2 * q8
                        elif qtg == 5:
                            # late half warms the PE through the last scores
                            s = base + 26 + 2 * q8 if q8 < 4 else base + 31 + 2 * q8
                        else:
                            # qtg6's quanta run after the last scores
                            s = base + 32 + q8
                        at(s, 4, outproj_mm(qtg, tt, nt))
                        q8 += 1
                at(base + {5: 47, 6: 41}.get(qtg, 42), 5, outproj_dma(qtg))

            # ---- projection / load filler (placed by deadline) ----
            # b0: scores(qt0, j) at step j needs kt jlet j; vnat g by pv step
            for j in range(1, NKT):
                at(j - 1, 4, proj_jlet(kt_sb[0], wk_sb, "kv", 0, j))
            for j in range(2, NKT):
                at(j - 2, 5, proj_jlet(vt_sb[0], wv_sb, "kv", 0, j))
            for g in range(4):
                at(4 * g + 5, 5, vnat_group(0, g))
            at(0, 0, lambda: nc.gpsimd.dma_start(wout_sb[:], wout_d.ap()))
            at(2, 0, load_x("kv", 0, 2, xkvt))
            at(4, 0, load_x("q", 0, 1, xqt))
            at(6, 0, load_x("kv", 0, 3, xkvt))
            at(18, 0, load_x("q", 0, 2, xqt))
            at(34, 0, load_x("q", 0, 3, xqt))
            for sub in range(4):
                at(11 + sub, 4, proj_jlet(qt_sb[0], wq_sb, "q", 0, 4 + sub))
            for sub in range(4):
                at(23 + 2 * sub, 4, proj_jlet(qt_sb[0], wq_sb, "q", 0, 8 + sub))
                at(35 + 2 * sub, 4, proj_jlet(qt_sb[0], wq_sb, "q", 0, 12 + sub))
            # b1 loads + projections, woven through b0's steady state
            for c in range(NCPB):
                at(24 + 8 * c, 0, load_x("kv", 1, c, xkvt))
            for j in range(NKT):
                at(28 + 2 * j, 4, proj_jlet(kt_sb[1], wk_sb, "kv", 1, j))
            for j in range(NKT):
                at(44 + j, 5, proj_jlet(vt_sb[1], wv_sb, "kv", 1, j))
            for g in range(4):
                at(60 + g, 4, vnat_group(1, g))
            for c in range(NCPB):
                at(48 + 14 * c, 0, load_x("q", 1, c, xqt))
                for sub in range(4):
                    s = 56 + sub if c == 0 else 54 + 16 * c + 2 * sub
                    at(s, 4, proj_jlet(qt_sb[1], wq_sb, "q", 1, 4 * c + sub))

            # ---------------- prologue: one DGE ring, strict priority order
            # (splitting rings splits the fixed aggregate bandwidth away
            # from the critical kv00+q00 prefix)
            load_x("q", 0, 0, xqt)()
            load_x("kv", 0, 0, xkvt)()
            load_x("kv", 0, 1, xkvt)()
            # spin the PE on junk matmuls while the first chunks are in
            # flight: keeps the clock at full p-state so the prologue
            # projections don't run cold.  The source tile is DVE-memset so
            # the spin starts right after the startup barrier (~7us), not
            # after the gpsimd identity setup (~12us).
            wsrc = persist.tile([128, 32], BF16, tag="wsrc")
            nc.vector.memset(wsrc[:], 0.25)
            warm_ps = miscp.tile([128, 32], F32, tag="m", name="warm")
            for _ in range(220):
                nc.tensor.matmul(
                    warm_ps[0:32, :], wsrc[:], wsrc[:],
                    start=True, stop=True,
                )
            for sub in range(4):
                proj_jlet(qt_sb[0], wq_sb, "q", 0, sub)()
            proj_jlet(kt_sb[0], wk_sb, "kv", 0, 0)()
            at(0, 3, proj_jlet(vt_sb[0], wv_sb, "kv", 0, 0))
            at(0, 3, proj_jlet(vt_sb[0], wv_sb, "kv", 0, 1))

            # ---------------- run the agenda ----------------
            for s in sorted(agenda):
                for _, fn in sorted(agenda[s], key=lambda pf: pf[0]):
                    fn()

            # ---------------- epilogue: last q-tile group ----------------
            qtg = B * NQT - 1
            b, qt = divmod(qtg, NQT)
            obs[qtg] = ost.tile([128, NQS, DIM], BF16, tag="o", name="ob")
            oTs = {}
            for h in range(HPC):
                oTs[h] = accp.tile([DH, NQS, 128], BF16, tag="acc", name="oT")
            for qs in range(NQS):
                for h in range(HPC):
                    nc.tensor.transpose(
                        oTs[h][:, qs, :], obufs[(qtg, h)][:, qs, :], ident128[:]
                    )
                    nc.vector.tensor_copy(
                        outT[b][
                            h * DH : (h + 1) * DH,
                            qt * QTILE + qs * 128 : qt * QTILE + (qs + 1) * 128,
                        ],
                        oTs[h][:, qs, :],
                    )
                t_sl = slice((qt * NQS + qs) * 128, (qt * NQS + qs + 1) * 128)
                for nt in range(2):
                    ps = miscp.tile([128, 512], F32, tag="m", name="projo")
                    for qq in range(4):
                        nc.tensor.matmul(
                            ps[:, qq * 128 : (qq + 1) * 128],
                            outT[b][:, t_sl],
                            wout_sb[:, nt * 512 + qq * 128 : nt * 512 + (qq + 1) * 128],
                            start=(qq == 0),
                            stop=(qq == 3),
                        )
                    dst = obs[qtg][:, qs, nt * 512 : (nt + 1) * 512]
                    if nt == 0:
                        nc.scalar.copy(dst, ps[:])
                    else:
                        nc.vector.tensor_copy(dst, ps[:])
                nc.gpsimd.dma_start(
                    out_d.ap()[qtg].rearrange("p t d -> t p d")[qs],
                    obs[qtg][:, qs, :],
                )
            obs.pop(qtg)

    nc.compile()
    return nc


def make_in_maps(x_q, x_kv, W_qkv, W_out):
    x_q = np.asarray(x_q, dtype=np.float32)
    x_kv = np.asarray(x_kv, dtype=np.float32)
    W_qkv = np.asarray(W_qkv, dtype=np.float32)
    W_out = np.asarray(W_out, dtype=np.float32)

    def chunk_tile(x):
        # [TOK, DIM] -> [n_chunks, 128, 4, KO, 128]: piece-contiguous layout,
        # token = c*512 + piece*128 + t, D = ko*128 + partition
        xt = x.reshape(TOK, DIM).T.reshape(KO, 128, TOK // PCHUNK, PCHUNK // KTILE, KTILE)
        return np.ascontiguousarray(xt.transpose(2, 1, 3, 0, 4)).astype(BF)

    xqt = chunk_tile(x_q)
    xkvt = chunk_tile(x_kv)

    in_maps = []
    for c in range(N_CORES):
        cs = slice(c * HD, (c + 1) * HD)
        in_maps.append(
            {
                "xqt": xqt,
                "xkvt": xkvt,
                "wq": np.ascontiguousarray(W_qkv[:, cs]).astype(BF),
                "wk": np.ascontiguousarray(W_qkv[:, 1024:][:, cs]).astype(BF),
                "wv": np.ascontiguousarray(W_qkv[:, 2048:][:, cs]).astype(BF),
                "wout": np.ascontiguousarray(W_out[cs, :]).astype(BF),
            }
        )
    return in_maps


def combine(partials, b_out):
    """Sum the 8 per-core partial projections and add the bias."""
    acc = np.zeros((B * NQT, 128, NQS, DIM), dtype=np.float32)
    for p in partials:
        acc += np.asarray(p, dtype=np.float32)
    # [bq, p, tt, d] -> token (bq, tt, p)
    acc = acc.transpose(0, 2, 1, 3).reshape(B, SQ, DIM)
    acc += np.asarray(b_out, dtype=np.float32)
    return acc


_STATE = {}


def _get_nc():
    if "nc" not in _STATE:
        _STATE["nc"] = build()
    return _STATE["nc"]


def run(x_q, x_kv, W_qkv, W_out, b_out, trace=False):
    nc = _get_nc()
    in_maps = make_in_maps(x_q, x_kv, W_qkv, W_out)
    res = run_bass_kernel_spmd(nc, in_maps, list(range(N_CORES)), trace=trace)
    out = combine([r["out"] for r in res.results], b_out)
    return out, res


def kernel(x_q, x_kv, W_qkv, W_out, b_out):
    out, _ = run(x_q, x_kv, W_qkv, W_out, b_out, trace=False)
    return out
